# revision 13
# baseline (speedup 1.0000x reference)
"""2x nearest-neighbor upsample of complex (real+imag) NHWC images on 8 trn2 cores.

out[t, b, i, j, c] = x_t[b, i // 2, j // 2, c]   (t = real/imag)

Strategy (data-parallel over batch, 2 images per core):
  - 4 full-image HWDGE loads pipelined on SP lanes 0/1 (depth 2), plus 8
    tiny "tail" loads that place the last T=8 input-w columns of the 8
    slow rows on helper partitions (see below)
  - per half-image chunk: one DVE broadcast copy expands W 2x into a
    [128, 8192] tile; stores duplicate output rows 2i/2i+1 via a
    0-stride broadcast middle dim in the DRAM AP (one DMA per region)
  - SDMA engine 15 (serving partitions {92-95, 124-127} by the port
    swizzle) runs ~11% slower than the other 15 engines on this part.
    With uniform layout it is the critical path. The kernel sheds the
    expanded tail (T=8 of 64 input-w cols, 12.5% of store bytes) of
    those 8 rows onto 8 helper partitions {0,4,...,28} that sit on 8
    distinct fast ports, equalizing per-engine finish times.
HBM traffic per core = 16 MiB read (+64 KiB duplicated tail reads)
+ 64 MiB write; the schedule keeps all 16 SDMA engines busy end-to-end.
"""

import sys

import numpy as np

if "/opt/trn_rl_repo" not in sys.path:
    sys.path.insert(0, "/opt/trn_rl_repo")

import concourse.bass as bass
import concourse.bass_isa as bass_isa
import concourse.mybir as mybir
import concourse.tile_sem_assignment as _tsa
from concourse.bass_utils import run_bass_kernel_spmd
from concourse.tile import TileContext
from concourse.tile_rust import add_dep_helper

# Partition HWDGE DMA-completion semaphore lanes by issuing engine: SP
# (loads) alternates lanes 0/1 (two loads in flight — Tile serializes
# DMAs within one lane via the own-lane-predecessor wait), ACT (stores)
# round-robins lanes 2-7. Each lane then carries DMAs from a single
# HWDGE FIFO ring, keeping every DMA at the 1 sync-wait walrus allows.
_orig_assign_tick = _tsa.TileClockTick._assign_tick


def _assign_tick_lane_split(self, inst):
    if isinstance(inst, _tsa.DMAInst) and not isinstance(
        inst, bass_isa.UserSyncedRemoteDMADescs
    ):
        if inst.engine == mybir.EngineType.Pool:
            self.next_sw_dma_idx = 0
        elif inst.engine == mybir.EngineType.SP:
            r = getattr(self, "_sp_lane_rr", 0)
            self.next_hw_dma_idx = r
            self._sp_lane_rr = (r + 1) % 2
        elif inst.engine == mybir.EngineType.Activation:
            r = getattr(self, "_act_lane_rr", 0)
            self.next_hw_dma_idx = 2 + r
            self._act_lane_rr = (r + 1) % 6
    return _orig_assign_tick(self, inst)


_tsa.TileClockTick._assign_tick = _assign_tick_lane_split

F32 = mybir.dt.float32

B, H, W, C = 16, 128, 128, 64
N_CORES = 8
BPC = B // N_CORES  # images per core
N_IMG = 2 * BPC  # (tensor, image) pairs per core

WK = W // 2  # input W columns per chunk (half an image row)
T = 8  # input-w columns shed from slow partitions (per half)
KEEP = WK - T
IMG_LEN = W * C  # 8192 f32 per partition per full-image tile
EXP = 2 * WK * C  # expanded chunk = 8192 f32 = 32 KB
EXPK = 2 * KEEP * C  # main-store cols = 7168 f32
TAILE = 2 * T * C  # expanded tail = 1024 f32
TIN_LEN = 2 * T * C  # tail-load payload per slow row = 2 halves * T * C

# slow rows (SDMA port 15) and their helper partitions (8 distinct fast
# ports: {0,4,...,28} hit ports 0,2,4,...,14)
SLOW_A = (92, 96)  # rows 92..95  -> helpers 0,4,8,12
SLOW_B = (124, 128)  # rows 124..127 -> helpers 16,20,24,28


def _build() -> bass.Bass:
    nc = bass.Bass("TRN2", debug=False)
    xr = nc.dram_tensor("x_real", [BPC, H, W, C], F32, kind="ExternalInput").ap()
    xi = nc.dram_tensor("x_imag", [BPC, H, W, C], F32, kind="ExternalInput").ap()
    out = nc.dram_tensor(
        "out", [2, BPC, 2 * H, 2 * W, C], F32, kind="ExternalOutput"
    ).ap()

    # walrus codegen allows exactly ONE sync-wait command per engine
    # instruction. Tile emits a wait only when the issuing engine has not
    # already observed that semaphore tick through an earlier *real*
    # instruction's wait. Every instruction below is budgeted to observe
    # at most one fresh tick, using tiny "absorber" instructions
    # (1-element memsets on DVE, 2-element probe copies on ACT, 4-byte
    # writes on SP) to pre-observe everything else.
    with TileContext(nc) as tc:
        with (
            tc.tile_pool(name="pin", bufs=3) as pin,
            tc.tile_pool(name="ptin", bufs=N_IMG) as ptin,
            tc.tile_pool(name="pout", bufs=2) as pout,
            tc.tile_pool(name="ptout", bufs=2) as ptout,
            tc.tile_pool(name="pdummy", bufs=1) as pdummy,
        ):
            adummy = pdummy.tile([1, 4 * 8 + 2 * N_IMG], F32, name="adummy")
            vdummy = pdummy.tile([1, 12 * 8], F32, name="vdummy")
            spdummy = pdummy.tile([1, 16], F32, name="spdummy")
            nv = 0  # vdummy cursor

            imgs = [(x, b) for x in (xr, xi) for b in range(BPC)]

            def tail_loads(i):
                """One shared [32, TIN_LEN] tile per image; rows 92-95 land
                on partitions 0,4,8,12 and rows 124-127 on 16,20,24,28
                (strided single-dim partition APs only — nests are broken)."""
                x, b = imgs[i]
                tt = ptin.tile([32, TIN_LEN], F32, name="ttin")
                xv = x[b].rearrange("i (h v) c -> i h v c", h=2)
                dst = tt.rearrange("p (h w c) -> p h w c", h=2, c=C)
                lda = nc.sync.dma_start(
                    out=dst[0:16:4], in_=xv[SLOW_A[0] : SLOW_A[1], :, KEEP:WK, :]
                )
                ldb = nc.sync.dma_start(
                    out=dst[16:32:4], in_=xv[SLOW_B[0] : SLOW_B[1], :, KEEP:WK, :]
                )
                return tt, lda, ldb

            # Load prologue: images 0-2 into dedicated tin slots; image 3
            # reuses slot 0 (emitted later, after its WAR targets exist).
            tins = [None] * N_IMG
            ld_ms = [None] * N_IMG
            tts = [None] * N_IMG
            ld_tas = [None] * N_IMG
            ld_tbs = [None] * N_IMG

            tts[0], ld_tas[0], ld_tbs[0] = tail_loads(0)
            for i in (0, 1):
                tins[i] = pin.tile([H, IMG_LEN], F32, name="tin")
                ld_ms[i] = nc.sync.dma_start(out=tins[i][:, :], in_=imgs[i][0][imgs[i][1]])
            tts[1], ld_tas[1], ld_tbs[1] = tail_loads(1)
            tins[2] = pin.tile([H, IMG_LEN], F32, name="tin")
            ld_ms[2] = nc.sync.dma_start(out=tins[2][:, :], in_=imgs[2][0][imgs[2][1]])

            copy_mains = [None] * 8
            copy_tbs = [None] * N_IMG
            probe1s = [None] * 8
            probe2s = [None] * N_IMG
            st_all = []  # ACT store DMAs in emission order (lane = 2 + idx%6)
            st_mains = [None] * 8
            st_f1s = [None] * 8
            st_f2s = [None] * 8
            st_has = [None] * 8
            st_hbs = [None] * 8
            ttouts = [None] * N_IMG

            for j in range(8):
                i_img, h = divmod(j, 2)
                x, b = imgs[i_img]
                t = 0 if x is xr else 1
                ov = out[t, b].rearrange("(i r) w c -> i r (w c)", r=2)
                tout = pout.tile([H, EXP], F32, name="tout")

                if j == 4:
                    # image 3 reuses tin slot 0 (copies 0,1 read it).
                    # Only a REAL engine instruction's wait counts as
                    # "observed" (SP Write is sequencer-only), so issue
                    # this one load from gpsimd/SWDGE with a Pool memset
                    # absorbing the DVE tick — the latest copy's tick
                    # covers the slot-release bundle.
                    pabs = reuse_pabs = nc.gpsimd.memset(spdummy[:1, 15:16], 0.0)
                    add_dep_helper(
                        pabs.ins, copy_mains[3].ins, sync=True,
                        reason="Pool observes DVE for tin slot reuse",
                    )
                    tins[3] = pin.tile([H, IMG_LEN], F32, name="tin")
                    ld_ms[3] = nc.gpsimd.dma_start(
                        out=tins[3][:, :], in_=imgs[3][0][imgs[3][1]]
                    )
                    add_dep_helper(
                        ld_ms[3].ins, pabs.ins, sync=False,
                        reason="absorber runs before reuse load",
                    )
                    tts[2], ld_tas[2], ld_tbs[2] = tail_loads(2)
                    tts[3], ld_tas[3], ld_tbs[3] = tail_loads(3)

                # --- DVE absorbers for this chunk's slot recycling ---
                vabss = []

                def vabs(dep, reason):
                    nonlocal nv
                    m = nc.vector.memset(vdummy[:1, nv : nv + 1], 0.0)
                    nv += 1
                    add_dep_helper(m.ins, dep.ins, sync=True, reason=reason)
                    vabss.append(m)

                if j >= 2:
                    # watermark: observing the previous copy's own-engine
                    # tick (trivially satisfied) pre-observes all pool
                    # slot-release ticks on the DVE sem.
                    vabs(copy_mains[j - 1], "DVE self-sem watermark")
                    vabs(probe1s[j - 2], "absorb probe WAR (ACT sem)")
                    vabs(st_mains[j - 2], "absorb tout WAR (main lane)")
                    vabs(st_f1s[j - 2], "absorb tout WAR (f1 lane)")
                    vabs(st_f2s[j - 2], "absorb tout WAR (f2 lane)")
                if h == 0 and i_img >= 2:
                    for k in (2 * (i_img - 2), 2 * (i_img - 2) + 1):
                        vabs(st_has[k], "absorb ttout WAR (ha lane)")
                        vabs(st_hbs[k], "absorb ttout WAR (hb lane)")

                # --- DVE copies ---
                cps = []
                if h == 0:
                    # one combined tail copy over partitions [0:32) (DVE
                    # partition base must be 0): depends on BOTH tail
                    # loads, so absorb the first load's lane tick.
                    vabs(ld_tas[i_img], "absorb tail-load-a lane tick")
                    ttout = ptout.tile([32, 2 * TAILE], F32, name="ttout")
                    ttouts[i_img] = ttout
                    tsrc = tts[i_img].rearrange(
                        "p (g w c) -> p g w c", g=2, c=C
                    ).unsqueeze(3).broadcast_to([32, 2, T, 2, C])
                    tdst = ttout.rearrange(
                        "p (g w s c) -> p g w s c", g=2, s=2, c=C
                    )
                    cpt = nc.vector.tensor_copy(out=tdst, in_=tsrc)
                    copy_tbs[i_img] = cpt
                    cps.append(cpt)
                src = (
                    tins[i_img][:, h * WK * C : (h + 1) * WK * C]
                    .rearrange("p (w c) -> p w c", c=C)
                    .unsqueeze(2)
                    .broadcast_to([H, WK, 2, C])
                )
                dst = tout.rearrange("p (w s c) -> p w s c", s=2, c=C)
                cp = nc.vector.tensor_copy(out=dst, in_=src)
                copy_mains[j] = cp
                cps.append(cp)
                for vb in vabss:
                    for c_ in cps:
                        add_dep_helper(
                            c_.ins, vb.ins, sync=False,
                            reason="absorbers run before copies",
                        )

                # --- ACT probes (absorb the DVE data waits) ---
                if h == 0:
                    p2 = nc.scalar.copy(
                        out=adummy[:1, 32 + 2 * i_img : 34 + 2 * i_img],
                        in_=ttouts[i_img][0:1, 0:2],
                    )
                    probe2s[i_img] = p2
                p1 = nc.scalar.copy(
                    out=adummy[:1, 4 * j : 4 * j + 2], in_=tout[:1, 0:2]
                )
                probe1s[j] = p1

                # --- stores (middle dim r broadcast: rows 2i and 2i+1) ---
                o0 = h * EXP

                def store(out_ap, in_ap, probe):
                    st = nc.scalar.dma_start(out=out_ap, in_=in_ap)
                    add_dep_helper(
                        st.ins, probe.ins, sync=False,
                        reason="probe runs before store",
                    )
                    st_all.append(st)
                    return st

                st_mains[j] = store(
                    ov[:, :, o0 : o0 + EXPK],
                    tout[:, :EXPK].unsqueeze(1).broadcast_to([H, 2, EXPK]),
                    p1,
                )
                st_f1s[j] = store(
                    ov[0 : SLOW_A[0], :, o0 + EXPK : o0 + EXP],
                    tout[0 : SLOW_A[0], EXPK:EXP]
                    .unsqueeze(1)
                    .broadcast_to([SLOW_A[0], 2, TAILE]),
                    p1,
                )
                st_f2s[j] = store(
                    ov[SLOW_A[1] : SLOW_B[0], :, o0 + EXPK : o0 + EXP],
                    tout[SLOW_A[1] : SLOW_B[0], EXPK:EXP]
                    .unsqueeze(1)
                    .broadcast_to([SLOW_B[0] - SLOW_A[1], 2, TAILE]),
                    p1,
                )
                tto = ttouts[i_img]
                st_has[j] = store(
                    ov[SLOW_A[0] : SLOW_A[1], :, o0 + EXPK : o0 + EXP],
                    tto[0:16:4, h * TAILE : (h + 1) * TAILE]
                    .unsqueeze(1)
                    .broadcast_to([4, 2, TAILE]),
                    probe2s[i_img],
                )
                st_hbs[j] = store(
                    ov[SLOW_B[0] : SLOW_B[1], :, o0 + EXPK : o0 + EXP],
                    tto[16:32:4, h * TAILE : (h + 1) * TAILE]
                    .unsqueeze(1)
                    .broadcast_to([4, 2, TAILE]),
                    probe2s[i_img],
                )

            # Kernel-tail absorbers: pre-observe every outstanding proc
            # with one 4-byte SP write per tick so the final drain lowers
            # to cheap 1-wait structs. Lanes 0/1 via the last tail loads,
            # lanes 2-7 via the last store on each, ACT via the last
            # probe, DVE via the last copy.
            last_per_lane = {}
            for idx, st in enumerate(st_all):
                last_per_lane[2 + idx % 6] = (idx, st)
            lane_sts = [v for v in last_per_lane.values()]
            lane_sts.sort(key=lambda kv: kv[0])
            tail_deps = (
                [ld_tas[3], ld_tbs[3], reuse_pabs, ld_ms[3], copy_mains[7],
                 probe1s[7]]
                + [st for _, st in lane_sts]
            )
            for j, dep in enumerate(tail_deps):
                wr = nc.sync.write(spdummy[:1, j : j + 1], b"\x00\x00\x00\x00")
                add_dep_helper(
                    wr.ins, dep.ins, sync=True,
                    reason="pre-observe outstanding procs for tail drain",
                )
    return nc


_NC_CACHE: bass.Bass | None = None


def _get_nc() -> bass.Bass:
    global _NC_CACHE
    if _NC_CACHE is None:
        _NC_CACHE = _build()
    return _NC_CACHE


def _run(x_real: np.ndarray, x_imag: np.ndarray, **spmd_kwargs):
    x_real = np.ascontiguousarray(np.asarray(x_real, dtype=np.float32))
    x_imag = np.ascontiguousarray(np.asarray(x_imag, dtype=np.float32))
    assert x_real.shape == (B, H, W, C), x_real.shape
    assert x_imag.shape == (B, H, W, C), x_imag.shape
    in_maps = [
        {
            "x_real": x_real[c * BPC : (c + 1) * BPC],
            "x_imag": x_imag[c * BPC : (c + 1) * BPC],
        }
        for c in range(N_CORES)
    ]
    res = run_bass_kernel_spmd(
        _get_nc(), in_maps, core_ids=list(range(N_CORES)), **spmd_kwargs
    )
    full = np.concatenate([r["out"] for r in res.results], axis=1)
    return full, res


def kernel(x_real: np.ndarray, x_imag: np.ndarray) -> np.ndarray:
    full, _ = _run(x_real, x_imag)
    return full


# revision 14
# speedup vs baseline: 1.1408x; 1.1408x over previous
"""2x nearest-neighbor upsample of complex (real+imag) NHWC images on 8 trn2 cores.

out[t, b, i, j, c] = x_t[b, i // 2, j // 2, c]   (t = real/imag)

Strategy (data-parallel over batch, 2 images per core):
  - 4 full-image HWDGE loads into dedicated SBUF tiles, issued
    back-to-back at t=0 on SP completion-sem lanes 0/1 (Tile serializes
    DMAs within one lane via the own-lane-predecessor wait, so two lanes
    give two loads in flight)
  - per half-image chunk: ONE DVE broadcast copy expands W 2x into a
    [128, 8192] tile (partition i = input row i), then ONE 8 MiB HWDGE
    store writes both duplicated output rows 2i/2i+1 via a 0-stride
    broadcast middle dim on the SBUF side of the AP
  - only full-128-partition big-descriptor DMAs are used: those are the
    only shape the HWDGE spreads uniformly across all 16 SDMA engines
    (partial-partition or small-descriptor DMAs clump onto engines 0-3)
HBM traffic per core = 16 MiB read + 64 MiB write (the minimum); the
schedule keeps all 16 SDMA engines busy end-to-end.
"""

import sys

import numpy as np

if "/opt/trn_rl_repo" not in sys.path:
    sys.path.insert(0, "/opt/trn_rl_repo")

import concourse.bass as bass
import concourse.bass_isa as bass_isa
import concourse.mybir as mybir
import concourse.tile_sem_assignment as _tsa
from concourse.bass_utils import run_bass_kernel_spmd
from concourse.tile import TileContext
from concourse.tile_rust import add_dep_helper

# Partition HWDGE DMA-completion semaphore lanes by issuing engine: SP
# (loads) alternates lanes 0/1, ACT (stores) round-robins lanes 2-7.
# Each lane then carries DMAs from a single HWDGE FIFO ring (per-lane
# completion order is trivially sound), and a DMA's own-lane predecessor
# is always one the issuing engine has already observed — keeping every
# DMA at the 1 sync-wait walrus codegen allows.
_orig_assign_tick = _tsa.TileClockTick._assign_tick


def _assign_tick_lane_split(self, inst):
    if isinstance(inst, _tsa.DMAInst) and not isinstance(
        inst, bass_isa.UserSyncedRemoteDMADescs
    ):
        if inst.engine == mybir.EngineType.Pool:
            self.next_sw_dma_idx = 0
        elif inst.engine == mybir.EngineType.SP:
            r = getattr(self, "_sp_lane_rr", 0)
            self.next_hw_dma_idx = r
            self._sp_lane_rr = (r + 1) % 2
        elif inst.engine == mybir.EngineType.Activation:
            r = getattr(self, "_act_lane_rr", 0)
            self.next_hw_dma_idx = 2 + r
            self._act_lane_rr = (r + 1) % 6
    return _orig_assign_tick(self, inst)


_tsa.TileClockTick._assign_tick = _assign_tick_lane_split

F32 = mybir.dt.float32

B, H, W, C = 16, 128, 128, 64
N_CORES = 8
BPC = B // N_CORES  # images per core
N_IMG = 2 * BPC  # (tensor, image) pairs per core

WK = W // 2  # input W columns per chunk (half an image row)
IMG_LEN = W * C  # 8192 f32 per partition per full-image tile
EXP = 2 * WK * C  # expanded chunk = 8192 f32 = 32 KB per partition


def _build() -> bass.Bass:
    nc = bass.Bass("TRN2", debug=False)
    xr = nc.dram_tensor("x_real", [BPC, H, W, C], F32, kind="ExternalInput").ap()
    xi = nc.dram_tensor("x_imag", [BPC, H, W, C], F32, kind="ExternalInput").ap()
    out = nc.dram_tensor(
        "out", [2, BPC, 2 * H, 2 * W, C], F32, kind="ExternalOutput"
    ).ap()

    # walrus codegen allows exactly ONE sync-wait command per engine
    # instruction. Tile emits a wait only when the issuing engine has not
    # already observed that semaphore tick through an earlier *real*
    # instruction's wait. Every instruction below is budgeted to observe
    # at most one fresh tick, using tiny "absorber" instructions
    # (1-element memsets on DVE, 2-element probe copies on ACT, 4-byte
    # writes on SP) to pre-observe everything else.
    with TileContext(nc) as tc:
        with (
            tc.tile_pool(name="pin", bufs=N_IMG) as pin,
            tc.tile_pool(name="pout", bufs=2) as pout,
            tc.tile_pool(name="pdummy", bufs=1) as pdummy,
        ):
            adummy = pdummy.tile([1, 2 * 8], F32, name="adummy")
            vdummy = pdummy.tile([1, 3 * 8], F32, name="vdummy")
            spdummy = pdummy.tile([1, 16], F32, name="spdummy")

            imgs = [(x, b) for x in (xr, xi) for b in range(BPC)]

            # All loads first: dedicated tiles, no WAR/WAW deps -> SP
            # fires all four back-to-back; lanes 0/1 keep two in flight.
            tins = []
            loads = []
            for x, b in imgs:
                tin = pin.tile([H, IMG_LEN], F32, name="tin")
                loads.append(nc.sync.dma_start(out=tin[:, :], in_=x[b]))
                tins.append(tin)

            copies = [None] * 8
            probes = [None] * 8
            stores = [None] * 8
            for j in range(8):
                i_img, h = divmod(j, 2)
                x, b = imgs[i_img]
                t = 0 if x is xr else 1
                # partition i holds input row i, feeding output rows 2i, 2i+1
                ov = out[t, b].rearrange("(i r) w c -> i r (w c)", r=2)
                tout = pout.tile([H, EXP], F32, name="tout")
                # DVE-side absorbers: pre-observe the chunk j-2 tout slot
                # readers (its store's DMA lane + its ACT probe) and the
                # DVE self-sem slot-release tick, so the copy's only
                # fresh wait is its load's lane tick.
                vabss = []
                if j >= 2:
                    vabs0 = nc.vector.memset(vdummy[:1, 3 * j : 3 * j + 1], 0.0)
                    add_dep_helper(
                        vabs0.ins, copies[j - 1].ins, sync=True,
                        reason="DVE self-sem watermark (slot release)",
                    )
                    vabs1 = nc.vector.memset(
                        vdummy[:1, 3 * j + 1 : 3 * j + 2], 0.0
                    )
                    add_dep_helper(
                        vabs1.ins, stores[j - 2].ins, sync=True,
                        reason="absorb tout slot WAR (store lane)",
                    )
                    vabs2 = nc.vector.memset(
                        vdummy[:1, 3 * j + 2 : 3 * j + 3], 0.0
                    )
                    add_dep_helper(
                        vabs2.ins, probes[j - 2].ins, sync=True,
                        reason="absorb probe WAR (ACT sem)",
                    )
                    vabss = [vabs0, vabs1, vabs2]
                src = (
                    tins[i_img][:, h * WK * C : (h + 1) * WK * C]
                    .rearrange("p (w c) -> p w c", c=C)
                    .unsqueeze(2)
                    .broadcast_to([H, WK, 2, C])
                )
                dst = tout.rearrange("p (w s c) -> p w s c", s=2, c=C)
                cp = nc.vector.tensor_copy(out=dst, in_=src)
                copies[j] = cp
                for vb in vabss:
                    add_dep_helper(
                        cp.ins, vb.ins, sync=False,
                        reason="absorbers run before copy",
                    )
                # 2-element ACT probe of the copy's region absorbs the
                # DVE data wait; the store then fires with only its
                # own-lane-predecessor wait.
                probe = nc.scalar.copy(
                    out=adummy[:1, 2 * j : 2 * j + 2], in_=tout[:1, 0:2]
                )
                probes[j] = probe
                # one 8 MiB store: DRAM [row i][copy r: 2][8192 contig]
                # reads the same SBUF region twice via a 0-stride dim
                st = nc.scalar.dma_start(
                    out=ov[:, :, h * EXP : (h + 1) * EXP],
                    in_=tout.unsqueeze(1).broadcast_to([H, 2, EXP]),
                )
                add_dep_helper(
                    st.ins, probe.ins, sync=False,
                    reason="probe runs before store",
                )
                stores[j] = st

            # Kernel-tail absorbers: pre-observe every outstanding proc
            # with one 4-byte SP write per tick so the final drain lowers
            # to cheap 1-wait structs.
            tail_deps = (
                loads[-2:]  # lanes 0, 1
                + [copies[-1], probes[-1]]  # DVE, ACT
                + stores[2:]  # lanes 2-7 (last six stores cover all)
            )
            for j, dep in enumerate(tail_deps):
                wr = nc.sync.write(spdummy[:1, j : j + 1], b"\x00\x00\x00\x00")
                add_dep_helper(
                    wr.ins, dep.ins, sync=True,
                    reason="pre-observe outstanding procs for tail drain",
                )
    return nc


_NC_CACHE: bass.Bass | None = None


def _get_nc() -> bass.Bass:
    global _NC_CACHE
    if _NC_CACHE is None:
        _NC_CACHE = _build()
    return _NC_CACHE


def _run(x_real: np.ndarray, x_imag: np.ndarray, **spmd_kwargs):
    x_real = np.ascontiguousarray(np.asarray(x_real, dtype=np.float32))
    x_imag = np.ascontiguousarray(np.asarray(x_imag, dtype=np.float32))
    assert x_real.shape == (B, H, W, C), x_real.shape
    assert x_imag.shape == (B, H, W, C), x_imag.shape
    in_maps = [
        {
            "x_real": x_real[c * BPC : (c + 1) * BPC],
            "x_imag": x_imag[c * BPC : (c + 1) * BPC],
        }
        for c in range(N_CORES)
    ]
    res = run_bass_kernel_spmd(
        _get_nc(), in_maps, core_ids=list(range(N_CORES)), **spmd_kwargs
    )
    full = np.concatenate([r["out"] for r in res.results], axis=1)
    return full, res


def kernel(x_real: np.ndarray, x_imag: np.ndarray) -> np.ndarray:
    full, _ = _run(x_real, x_imag)
    return full


# revision 15
# speedup vs baseline: 1.2037x; 1.0551x over previous
"""2x nearest-neighbor upsample of complex (real+imag) NHWC images on 8 trn2 cores.

out[t, b, i, j, c] = x_t[b, i // 2, j // 2, c]   (t = real/imag)

Strategy (data-parallel over batch, 2 images per core):
  - prefetch the ENTIRE 16 MiB per-core input into 8 dedicated SBUF tiles
    (one per half-image chunk) via HWDGE loads alternating SP
    completion-sem lanes 0/1 — Tile serializes DMAs within one lane via
    the own-lane-predecessor wait, so two lanes keep two loads in
    flight; the loads carry no other dependencies and are never on the
    chunk-to-chunk critical path
  - per chunk: ONE DVE broadcast copy expands W 2x into a [128, 8192]
    tile (partition i = input row i), then TWO plain 4 MiB HWDGE stores
    on the ACT ring write duplicated output rows 2i and 2i+1 from the
    SAME region (a single store with a 0-stride broadcast dim measures
    ~18% slower per SDMA engine; two plain stores hit full rate)
  - no SWDGE traffic, no partial-partition or small-descriptor DMAs:
    only full-128-partition big-descriptor transfers spread uniformly
    across all 16 SDMA engines
HBM traffic per core = 16 MiB read + 64 MiB write (the minimum); the
schedule keeps all 16 SDMA engines busy end-to-end.
"""

import sys

import numpy as np

if "/opt/trn_rl_repo" not in sys.path:
    sys.path.insert(0, "/opt/trn_rl_repo")

import concourse.bass as bass
import concourse.bass_isa as bass_isa
import concourse.mybir as mybir
import concourse.tile_sem_assignment as _tsa
from concourse.bass_utils import run_bass_kernel_spmd
from concourse.tile import TileContext
from concourse.tile_rust import add_dep_helper

# Partition HWDGE DMA-completion semaphore lanes by issuing engine: SP
# (loads) alternates lanes 0/1, ACT (stores) round-robins lanes 2-7.
# Each lane then carries DMAs from a single HWDGE FIFO ring (per-lane
# completion order is trivially sound), and a DMA's own-lane predecessor
# is always one the issuing engine has already observed — keeping every
# DMA at the 1 sync-wait walrus codegen allows.
_orig_assign_tick = _tsa.TileClockTick._assign_tick


def _assign_tick_lane_split(self, inst):
    if isinstance(inst, _tsa.DMAInst) and not isinstance(
        inst, bass_isa.UserSyncedRemoteDMADescs
    ):
        if inst.engine == mybir.EngineType.Pool:
            self.next_sw_dma_idx = 0
        elif inst.engine == mybir.EngineType.SP:
            r = getattr(self, "_sp_lane_rr", 0)
            self.next_hw_dma_idx = r
            self._sp_lane_rr = (r + 1) % 2
        elif inst.engine == mybir.EngineType.Activation:
            r = getattr(self, "_act_lane_rr", 0)
            self.next_hw_dma_idx = 2 + r
            self._act_lane_rr = (r + 1) % 6
    return _orig_assign_tick(self, inst)


_tsa.TileClockTick._assign_tick = _assign_tick_lane_split

F32 = mybir.dt.float32

B, H, W, C = 16, 128, 128, 64
N_CORES = 8
BPC = B // N_CORES  # images per core

WH = W // 2  # input W columns per chunk (half an image row)
IN_LEN = WH * C  # 4096 f32 = 16 KB per partition
EXP = 2 * IN_LEN  # expanded chunk (W doubled) = 8192 f32 = 32 KB
N_CHUNKS = 2 * BPC * 2  # (tensor, image, half) = 8


def _build() -> bass.Bass:
    nc = bass.Bass("TRN2", debug=False)
    xr = nc.dram_tensor("x_real", [BPC, H, W, C], F32, kind="ExternalInput").ap()
    xi = nc.dram_tensor("x_imag", [BPC, H, W, C], F32, kind="ExternalInput").ap()
    out = nc.dram_tensor(
        "out", [2, BPC, 2 * H, 2 * W, C], F32, kind="ExternalOutput"
    ).ap()

    # walrus codegen allows exactly ONE sync-wait command per engine
    # instruction. Tile emits a wait only when the issuing engine has not
    # already observed that semaphore tick through an earlier *real*
    # instruction's wait. Every instruction below is budgeted to observe
    # at most one fresh tick, using tiny "absorber" instructions
    # (1-element memsets on DVE, 2-element probe copies on ACT, 4-byte
    # writes on SP) to pre-observe everything else.
    with TileContext(nc) as tc:
        with (
            tc.tile_pool(name="pin", bufs=N_CHUNKS) as pin,
            tc.tile_pool(name="pout", bufs=2) as pout,
            tc.tile_pool(name="pdummy", bufs=1) as pdummy,
        ):
            adummy = pdummy.tile([1, 2 * N_CHUNKS], F32, name="adummy")
            vdummy = pdummy.tile([1, 4 * N_CHUNKS], F32, name="vdummy")
            spdummy = pdummy.tile([1, 16], F32, name="spdummy")

            chunks = [
                (x, b, h)
                for x in (xr, xi)
                for b in range(BPC)
                for h in range(2)
            ]

            # All loads first: dedicated tiles, no WAR/WAW deps -> the SP
            # engine fires all 8 back-to-back; lanes 0/1 alternate so two
            # are always in flight on the SP HWDGE ring.
            tins = []
            loads = []
            for x, b, h in chunks:
                tin = pin.tile([H, IN_LEN], F32, name="tin")
                ld = nc.sync.dma_start(
                    out=tin[:, :], in_=x[b, :, h * WH : (h + 1) * WH, :]
                )
                tins.append(tin)
                loads.append(ld)

            probes = []
            st_los = []
            st_his = []
            copies = []
            for j, (x, b, h) in enumerate(chunks):
                t = 0 if x is xr else 1
                # partition i holds input row i, feeding output rows 2i, 2i+1
                ov = out[t, b].rearrange("(i r) w c -> i r (w c)", r=2)
                tout = pout.tile([H, EXP], F32, name="tout")
                if j >= 2:
                    # DVE self-sem watermark: observing the previous
                    # copy's own-engine tick (trivially satisfied by
                    # program order) pre-observes the chunk j-2 tout
                    # slot-release tick, so the copy needs no self-wait.
                    vabs0 = nc.vector.memset(vdummy[:1, 4 * j : 4 * j + 1], 0.0)
                    add_dep_helper(
                        vabs0.ins, copies[j - 1].ins, sync=True,
                        reason="DVE self-sem watermark (slot release)",
                    )
                    vabs1 = nc.vector.memset(
                        vdummy[:1, 4 * j + 1 : 4 * j + 2], 0.0
                    )
                    add_dep_helper(
                        vabs1.ins, st_los[j - 2].ins, sync=True,
                        reason="absorb tout slot WAR (store-lo lane)",
                    )
                    vabs2 = nc.vector.memset(
                        vdummy[:1, 4 * j + 2 : 4 * j + 3], 0.0
                    )
                    add_dep_helper(
                        vabs2.ins, st_his[j - 2].ins, sync=True,
                        reason="absorb tout slot WAR (store-hi lane)",
                    )
                    vabs3 = nc.vector.memset(
                        vdummy[:1, 4 * j + 3 : 4 * j + 4], 0.0
                    )
                    add_dep_helper(
                        vabs3.ins, probes[j - 2].ins, sync=True,
                        reason="absorb probe WAR (ACT sem)",
                    )
                    vabss = (vabs0, vabs1, vabs2, vabs3)
                else:
                    vabss = ()
                src = (
                    tins[j]
                    .rearrange("p (w c) -> p w c", c=C)
                    .unsqueeze(2)
                    .broadcast_to([H, WH, 2, C])
                )
                dst = tout.rearrange("p (w s c) -> p w s c", s=2, c=C)
                cp = nc.vector.tensor_copy(out=dst, in_=src)
                for vb in vabss:
                    add_dep_helper(
                        cp.ins, vb.ins, sync=False,
                        reason="absorbers run before copy",
                    )
                copies.append(cp)
                # 2-element ACT probe of the copy's region absorbs the
                # DVE data wait; both stores then fire with only their
                # own-lane-predecessor wait.
                probe = nc.scalar.copy(
                    out=adummy[:1, 2 * j : 2 * j + 2], in_=tout[:1, 0:2]
                )
                probes.append(probe)
                o0 = h * EXP
                st_lo = nc.scalar.dma_start(
                    out=ov[:, 0, o0 : o0 + EXP], in_=tout[:, :]
                )
                add_dep_helper(
                    st_lo.ins, probe.ins, sync=False,
                    reason="probe runs before store",
                )
                st_hi = nc.scalar.dma_start(
                    out=ov[:, 1, o0 : o0 + EXP], in_=tout[:, :]
                )
                add_dep_helper(
                    st_hi.ins, probe.ins, sync=False,
                    reason="probe runs before store",
                )
                st_los.append(st_lo)
                st_his.append(st_hi)

            # Kernel-tail absorbers: pre-observe every outstanding proc
            # with one 4-byte SP write per tick so the final drain lowers
            # to cheap 1-wait structs: lanes 0/1 via the last two loads,
            # lanes 2-7 via the last six stores, ACT via the last probe,
            # DVE via the last copy.
            tail_deps = [
                loads[-2], loads[-1],
                st_los[-3], st_his[-3],
                copies[-1], probes[-1],
                st_los[-2], st_his[-2],
                st_los[-1], st_his[-1],
            ]
            for j, dep in enumerate(tail_deps):
                wr = nc.sync.write(spdummy[:1, j : j + 1], b"\x00\x00\x00\x00")
                add_dep_helper(
                    wr.ins, dep.ins, sync=True,
                    reason="pre-observe outstanding procs for tail drain",
                )
    return nc


_NC_CACHE: bass.Bass | None = None


def _get_nc() -> bass.Bass:
    global _NC_CACHE
    if _NC_CACHE is None:
        _NC_CACHE = _build()
    return _NC_CACHE


def _run(x_real: np.ndarray, x_imag: np.ndarray, **spmd_kwargs):
    x_real = np.ascontiguousarray(np.asarray(x_real, dtype=np.float32))
    x_imag = np.ascontiguousarray(np.asarray(x_imag, dtype=np.float32))
    assert x_real.shape == (B, H, W, C), x_real.shape
    assert x_imag.shape == (B, H, W, C), x_imag.shape
    in_maps = [
        {
            "x_real": x_real[c * BPC : (c + 1) * BPC],
            "x_imag": x_imag[c * BPC : (c + 1) * BPC],
        }
        for c in range(N_CORES)
    ]
    res = run_bass_kernel_spmd(
        _get_nc(), in_maps, core_ids=list(range(N_CORES)), **spmd_kwargs
    )
    full = np.concatenate([r["out"] for r in res.results], axis=1)
    return full, res


def kernel(x_real: np.ndarray, x_imag: np.ndarray) -> np.ndarray:
    full, _ = _run(x_real, x_imag)
    return full
